# revision 38
# baseline (speedup 1.0000x reference)
"""Trainium2 Bass kernel for CausalSelfAttention (B=4, T=2048, C=768, H=6, D=128)
with RoPE + QK-RMSNorm.  v5: bf16 datapath, GPSIMD rank-1 offload, phased.

Sharding: 8 cores = batch(4) x head-group(2, 3 heads each). Each core:
  - phase A: Q/K/V projections, RoPE + RMSNorm on Q/K (Sqrt act-table).
    Partition-dim sums and broadcasts for the norm run on GPSIMD
    (partition_all_reduce / partition_broadcast), not the PE -- the PE only
    does real matmuls + the RoPE half-swap permutation.
  - phase B: causal attention (Exp act-table) with scores transposed
    (S^T: T_k on partitions, T_q free), heads round-robin per k-chunk to
    hide exp latency; softmax denominator broadcast on GPSIMD; c_proj after
    each T_q tile.
  - host sums the two head-group partials per batch.
All SBUF tiles bf16 (2x DVE, half DMA bytes); PSUM f32. Activation table
loads: 2 total (Sqrt set in phase A, Exp set in phase B).
"""

import numpy as np
import ml_dtypes

_B, _T, _C, _H, _D = 4, 2048, 768, 6, 128
_HPG = 3            # heads per group
_HD = _HPG * _D     # 384, per-group head dims
_NT = 4             # T tiles of 512
_TW = 512           # tile width (T_q)
_NKC = _T // 128    # 16 k-chunks of 128
_NCB = _C // 128    # 6 c_in chunks
_EPS = 1e-15

_cached = {}


def _build_nc():
    from contextlib import ExitStack
    from concourse import bacc, tile, mybir, bass_isa

    f32 = mybir.dt.float32
    bf16 = mybir.dt.bfloat16
    Act = mybir.ActivationFunctionType
    Red = bass_isa.ReduceOp

    nc = bacc.Bacc("TRN2", target_bir_lowering=False, debug=False)

    xT = nc.dram_tensor("xT", (128, _NCB * _T), bf16, kind="ExternalInput").ap()
    wq = nc.dram_tensor("wq", (128, _NCB * _HD), bf16, kind="ExternalInput").ap()
    wk = nc.dram_tensor("wk", (128, _NCB * _HD), bf16, kind="ExternalInput").ap()
    wv = nc.dram_tensor("wv", (128, _NCB * _HD), bf16, kind="ExternalInput").ap()
    wo = nc.dram_tensor("wo", (128, _HPG * _C), bf16, kind="ExternalInput").ap()
    cs = nc.dram_tensor("cs", (128, 2 * _T), bf16, kind="ExternalInput").ap()
    cst = nc.dram_tensor("cst", (128, 3 * 128), bf16, kind="ExternalInput").ap()
    out = nc.dram_tensor("out", (128, _NT * 4 * _C), bf16, kind="ExternalOutput").ap()

    with tile.TileContext(nc) as tc, ExitStack() as ctx, \
            nc.allow_low_precision(reason="bf16 datapath; f32 psum accumulation"):
        # --- pools ---
        pc = ctx.enter_context(tc.tile_pool(name="pc", bufs=1))         # persistents
        pg = ctx.enter_context(tc.tile_pool(name="pg", bufs=3))         # rope/sq scratch
        pa = ctx.enter_context(tc.tile_pool(name="pa", bufs=12))        # A chunks
        psm = ctx.enter_context(tc.tile_pool(name="psm", bufs=3))       # small vectors
        pdb = ctx.enter_context(tc.tile_pool(name="pdb", bufs=2))       # bcast tiles
        pob = ctx.enter_context(tc.tile_pool(name="pob", bufs=2))       # out staging
        # psum pools (8 banks): proj/S x3, sw/O x3, den x1, c_proj x1
        ppS = ctx.enter_context(tc.tile_pool(name="ppS", bufs=3, space="PSUM"))
        ppO = ctx.enter_context(tc.tile_pool(name="ppO", bufs=3, space="PSUM"))
        ppM = ctx.enter_context(tc.tile_pool(name="ppM", bufs=1, space="PSUM"))
        ppB = ctx.enter_context(tc.tile_pool(name="ppB", bufs=1, space="PSUM"))

        # --- DMA inputs (host pre-packs layouts; all bf16), ordered by
        # first-use time: wk/x for K-projs, then wv/cst (V-projs + rope),
        # then cs (rope), wq (Q-projs), wo (c_proj, phase B) ---
        t_xt = pc.tile([128, _NCB * _T], bf16, tag="xt", name="t_xt")
        t_wk = pc.tile([128, _NCB * _HD], bf16, tag="wk", name="t_wk")
        t_wv = pc.tile([128, _NCB * _HD], bf16, tag="wv", name="t_wv")
        t_cs = pc.tile([128, 2 * _T], bf16, tag="cs", name="t_cs")
        t_cst = pc.tile([128, 3 * 128], bf16, tag="cst", name="t_cst")
        t_wq = pc.tile([128, _NCB * _HD], bf16, tag="wq", name="t_wq")
        t_wo = pc.tile([128, _HPG * _C], bf16, tag="wo", name="t_wo")
        nc.sync.dma_start(t_wk[:, 0:_HD], wk[:, 0:_HD])
        nc.sync.dma_start(t_xt[:, 0:_T], xT[:, 0:_T])
        nc.sync.dma_start(t_wk[:, _HD:], wk[:, _HD:])
        for c in range(1, 4):
            nc.sync.dma_start(t_xt[:, c * _T:(c + 1) * _T], xT[:, c * _T:(c + 1) * _T])
        nc.sync.dma_start(t_wv[:], wv[:])
        nc.sync.dma_start(t_cst[:], cst[:])
        for c in range(4, _NCB):
            nc.sync.dma_start(t_xt[:, c * _T:(c + 1) * _T], xT[:, c * _T:(c + 1) * _T])
        nc.sync.dma_start(t_cs[:], cs[:])
        nc.sync.dma_start(t_wq[:], wq[:])
        nc.sync.dma_start(t_wo[:], wo[:])
        t_eps = pc.tile([128, 1], f32, tag="eps", name="t_eps")
        nc.gpsimd.memset(t_eps[:], _EPS)

        t_tri = t_cst[:, 0:128]
        t_ones_col = t_cst[:, 128:129]
        t_perm = t_cst[:, 256:384]

        # persistent per-head K^T/Q^T (post rope+norm) and V blocks
        t_kn = [pc.tile([128, _T], bf16, tag=f"kn{h}", name=f"kn{h}") for h in range(_HPG)]
        t_qn = [pc.tile([128, _T], bf16, tag=f"qn{h}", name=f"qn{h}") for h in range(_HPG)]
        t_v = [pc.tile([128, _HD], bf16, tag=f"v{tb}", name=f"v{tb}") for tb in range(_NKC)]

        # ---------------- phase A emitters ----------------
        def proj_qk(dst, w, i, h):
            """dst[:, i*512:(i+1)*512] = (W x)^T tile for head h."""
            p = ppS.tile([128, _TW], f32, tag="pS", name="p_qk")
            for c in range(_NCB):
                nc.tensor.matmul(
                    p[:], w[:, c * _HD + h * 128: c * _HD + (h + 1) * 128],
                    t_xt[:, c * _T + i * _TW: c * _T + (i + 1) * _TW],
                    start=(c == 0), stop=(c == _NCB - 1))
            nc.scalar.copy(dst[:, i * _TW:(i + 1) * _TW], p[:])

        def proj_v(tb):
            p = ppS.tile([128, _HD], f32, tag="pS", name="p_v",
                         padded_shape=[128, _TW])
            for c in range(_NCB):
                nc.tensor.matmul(
                    p[:], t_xt[:, c * _T + tb * 128: c * _T + (tb + 1) * 128],
                    t_wv[:, c * _HD:(c + 1) * _HD],
                    start=(c == 0), stop=(c == _NCB - 1))
            nc.scalar.copy(t_v[tb][:], p[:])

        def rope_stage(dst_list, i):
            """In-place RoPE on dst[h][:, i*512:+512] for all heads."""
            isl = slice(i * _TW, (i + 1) * _TW)
            ssl = slice(_T + i * _TW, _T + (i + 1) * _TW)
            sws = []
            for h in range(_HPG):
                p_sw = ppO.tile([128, _TW], f32, tag="pO", name="p_sw")
                nc.tensor.matmul(p_sw[:], t_perm, dst_list[h][:, isl],
                                 start=True, stop=True)
                t_sw = pg.tile([128, _TW], bf16, tag="sw", name="t_sw", bufs=3)
                nc.vector.tensor_mul(t_sw[:], p_sw[:], t_cs[:, ssl])
                sws.append(t_sw)
            for h in range(_HPG):
                nc.vector.tensor_mul(dst_list[h][:, isl], dst_list[h][:, isl],
                                     t_cs[:, isl])
                nc.vector.tensor_add(dst_list[h][:, isl], dst_list[h][:, isl],
                                     sws[h][:])

        def norm_pre(dst_list, i):
            """RMSNorm front half: square (DVE) -> partition all-reduce
            (GPSIMD, 3 heads land in one (128,1536) tile) -> one batched
            sqrt (Act) + reciprocal (DVE) on the (1,1536) row -> per-head
            broadcast (GPSIMD). Returns the 3 broadcast tiles."""
            isl = slice(i * _TW, (i + 1) * _TW)
            t_ms = pg.tile([128, _HPG * _TW], bf16, tag="ms", name="t_ms",
                           bufs=2)
            for h in range(_HPG):
                t_sq = pg.tile([128, _TW], bf16, tag="sq", name="t_sq", bufs=3)
                nc.vector.tensor_mul(t_sq[:], dst_list[h][:, isl],
                                     dst_list[h][:, isl])
                nc.gpsimd.partition_all_reduce(
                    t_ms[:, h * _TW:(h + 1) * _TW], t_sq[:], 128, Red.add)
            t_sd = psm.tile([1, _HPG * _TW], bf16, tag="sd", name="t_sd", bufs=3)
            nc.scalar.activation(t_sd[:], t_ms[0:1, :], Act.Sqrt,
                                 bias=t_eps[0:1, :], scale=1.0 / 128.0)
            t_rs = psm.tile([1, _HPG * _TW], bf16, tag="rs", name="t_rs", bufs=3)
            nc.vector.reciprocal(t_rs[:], t_sd[:])
            bcs = []
            for h in range(_HPG):
                t_bc = pdb.tile([128, _TW], bf16, tag="bc", name="t_bc", bufs=3)
                nc.gpsimd.partition_broadcast(t_bc[:],
                                              t_rs[0:1, h * _TW:(h + 1) * _TW])
                bcs.append(t_bc)
            return bcs

        def norm_fin(dst_list, i, bcs):
            """RMSNorm back half: in-place scale (DVE, all-SBUF bf16 2x)."""
            isl = slice(i * _TW, (i + 1) * _TW)
            for h in range(_HPG):
                nc.vector.tensor_mul(dst_list[h][:, isl], dst_list[h][:, isl],
                                     bcs[h][:])

        # ---------------- phase B ----------------
        def attention(qt):
            """Causal attention for all 3 heads of one T_q tile, heads
            round-robin per k-chunk. Returns z tiles."""
            nchunk = 4 * qt + 4
            LOOKAHEAD = 3
            p_den = ppM.tile([65, _TW], f32, tag="pM", name="p_den")
            p_os = [ppO.tile([128, _TW], f32, tag="pO", name=f"p_o{h}")
                    for h in range(_HPG)]
            a_tiles = {}

            def emit_s(kc, h):
                roff = 0 if kc < 4 * qt else (kc - 4 * qt) * 128
                nsl = slice(roff, _TW)
                ksl = slice(kc * 128, (kc + 1) * 128)
                p_s = ppS.tile([128, _TW], f32, tag="pS", name="p_s")
                nc.tensor.matmul(p_s[:, nsl], t_kn[h][:, ksl],
                                 t_qn[h][:, qt * _TW + roff:(qt + 1) * _TW],
                                 start=True, stop=True)
                t_a = pa.tile([128, _TW], bf16, tag="a", name="t_a")
                nc.scalar.activation(t_a[:, nsl], p_s[:, nsl], Act.Exp,
                                     scale=1.0 / float(np.sqrt(_D)))
                if kc >= 4 * qt:  # diagonal chunk: triangular mask
                    dsl = slice(roff, roff + 128)
                    nc.gpsimd.tensor_mul(t_a[:, dsl], t_a[:, dsl], t_tri)
                a_tiles[(kc, h)] = t_a

            def emit_acc(kc, h):
                roff = 0 if kc < 4 * qt else (kc - 4 * qt) * 128
                nsl = slice(roff, _TW)
                t_a = a_tiles.pop((kc, h))
                nc.tensor.matmul(p_den[32 * h:32 * h + 1, nsl], t_ones_col,
                                 t_a[:, nsl],
                                 start=(kc == 0), stop=(kc == nchunk - 1))
                nc.tensor.matmul(p_os[h][:, nsl],
                                 t_v[kc][:, h * 128:(h + 1) * 128], t_a[:, nsl],
                                 start=(kc == 0), stop=(kc == nchunk - 1))

            for kc in range(nchunk + LOOKAHEAD):
                for h in range(_HPG):
                    if kc < nchunk:
                        emit_s(kc, h)
                    if kc >= LOOKAHEAD:
                        emit_acc(kc - LOOKAHEAD, h)
            # normalization: extract dens, PE broadcast (partition_broadcast
            # cannot read partition-offset rows), reciprocal, mul
            t_dn = psm.tile([65, _TW], bf16, tag="dn", name="t_dn", bufs=3)
            nc.scalar.copy(t_dn[:], p_den[:])
            zs = []
            for h in range(_HPG):
                p_db = ppB.tile([128, _TW], f32, tag="pB", name="p_db")
                nc.tensor.matmul(p_db[:], t_cst[32 * h:32 * h + 1, 128:256],
                                 t_dn[32 * h:32 * h + 1, :],
                                 start=True, stop=True)
                t_db = pdb.tile([128, _TW], bf16, tag="db", name="t_db")
                nc.vector.reciprocal(t_db[:], p_db[:])
                t_z = pc.tile([128, _TW], bf16, tag=f"z{h}_{qt % 2}",
                              name=f"z{h}_{qt % 2}")
                nc.vector.tensor_mul(t_z[:], p_os[h][:], t_db[:])
                zs.append(t_z)
            return zs

        def c_proj(qt, zs):
            t_ob = pob.tile([128, 4 * _C], bf16, tag="ob", name="t_ob")
            for blk in range(4):
                bsl = slice(blk * 128, (blk + 1) * 128)
                for nh in range(2):
                    p_c = ppB.tile([128, 384], f32, tag="pB", name="p_c",
                                   padded_shape=[128, _TW])
                    for hh in range(_HPG):
                        nc.tensor.matmul(
                            p_c[:], zs[hh][:, bsl],
                            t_wo[:, hh * _C + nh * 384: hh * _C + (nh + 1) * 384],
                            start=(hh == 0), stop=(hh == _HPG - 1))
                    o0 = blk * _C + nh * 384
                    nc.vector.tensor_copy(t_ob[:, o0:o0 + 384], p_c[:])
            nc.sync.dma_start(out[:, qt * 4 * _C:(qt + 1) * 4 * _C], t_ob[:])

        # ---------------- emission schedule ----------------
        # V-proj batches double as PE fillers inside the rope/norm windows;
        # norm halves are split so the DVE stream of the next stage is not
        # queued behind a GPSIMD/Act hop.
        for i in range(_NT):
            for h in range(_HPG):
                proj_qk(t_kn[h], t_wk, i, h)
        vq = list(range(_NKC))                   # V blocks yet to emit

        def v_fill(n):
            for tb in vq[:n]:
                proj_v(tb)
            del vq[:n]

        for i in range(_NT):
            rope_stage(t_kn, i)
            v_fill(2)                            # cover K-rope DVE
            bcs_k = norm_pre(t_kn, i)
            for h in range(_HPG):                # cover K reduce/sqrt/bcast
                proj_qk(t_qn[h], t_wq, i, h)
            norm_fin(t_kn, i, bcs_k)
            rope_stage(t_qn, i)
            v_fill(2)                            # cover Q-rope DVE
            bcs_q = norm_pre(t_qn, i)
            v_fill(1)                            # cover Q reduce/sqrt/bcast
            norm_fin(t_qn, i, bcs_q)
        v_fill(len(vq))
        for qt in range(_NT):
            zs = attention(qt)
            c_proj(qt, zs)

    nc.compile()
    return nc


def _get_nc():
    if "nc" not in _cached:
        _cached["nc"] = _build_nc()
    return _cached["nc"]


def _bf16(a):
    return np.ascontiguousarray(a.astype(ml_dtypes.bfloat16))


def make_in_maps(x, cos, sin, Wq, Wk, Wv, Wo):
    cosT = cos.reshape(_T, _D // 2).T                        # (64, T)
    sinT = sin.reshape(_T, _D // 2).T
    cc = np.concatenate([cosT, cosT], axis=0)                # (128, T)
    ss = np.concatenate([sinT, -sinT], axis=0)
    cs = _bf16(np.concatenate([cc, ss], axis=1))             # (128, 2T)
    tri = (np.arange(128)[None, :] >= np.arange(128)[:, None]).astype(np.float32)
    ones128 = np.ones((128, 128), dtype=np.float32)
    permm = np.zeros((128, 128), dtype=np.float32)           # half-swap permutation
    for d in range(64):
        permm[64 + d, d] = 1.0
        permm[d, 64 + d] = 1.0
    cst = _bf16(np.concatenate([tri, ones128, permm], axis=1))

    def pack_w(w):  # (768, 384) -> (128, 2304) c-chunk-major
        return _bf16(w.reshape(_NCB, 128, _HD).transpose(1, 0, 2).reshape(128, -1))

    def pack_wo(w):  # (384, 768) -> (128, 2304) head-chunk-major
        return _bf16(w.reshape(_HPG, 128, _C).transpose(1, 0, 2).reshape(128, -1))

    in_maps = []
    for core in range(8):
        b, g = divmod(core, 2)
        gsl = slice(g * _HD, (g + 1) * _HD)
        xt = x[b].T.reshape(_NCB, 128, _T).transpose(1, 0, 2).reshape(128, -1)
        in_maps.append({
            "xT": _bf16(xt),
            "wq": pack_w(Wq[gsl, :].T),
            "wk": pack_w(Wk[gsl, :].T),
            "wv": pack_w(Wv[gsl, :].T),
            "wo": pack_wo(Wo[:, gsl].T),
            "cs": cs, "cst": cst,
        })
    return in_maps


def unshard(core_outs):
    """core_outs: list of 8 per-core (128, 12288) bf16 arrays -> (4, 2048, 768)."""
    full = []
    for b in range(_B):
        acc = None
        for g in range(2):
            dev = np.asarray(core_outs[2 * b + g]).astype(np.float32)
            part = dev.reshape(128, _NT, 4, _C).transpose(1, 2, 0, 3).reshape(_T, _C)
            acc = part if acc is None else acc + part
        full.append(acc)
    return np.stack(full, axis=0)


def kernel(x, cos, sin, Wq, Wk, Wv, Wo):
    from concourse.bass_utils import run_bass_kernel_spmd

    x = np.asarray(x, dtype=np.float32)
    cos = np.asarray(cos, dtype=np.float32)
    sin = np.asarray(sin, dtype=np.float32)
    Wq = np.asarray(Wq, dtype=np.float32)
    Wk = np.asarray(Wk, dtype=np.float32)
    Wv = np.asarray(Wv, dtype=np.float32)
    Wo = np.asarray(Wo, dtype=np.float32)

    nc = _get_nc()
    in_maps = make_in_maps(x, cos, sin, Wq, Wk, Wv, Wo)
    res = run_bass_kernel_spmd(nc, in_maps, core_ids=list(range(8)))
    return unshard([r_["out"] for r_ in res.results])


# revision 53
# speedup vs baseline: 1.0482x; 1.0482x over previous
"""Trainium2 Bass kernel for CausalSelfAttention (B=4, T=2048, C=768, H=6, D=128)
with RoPE + QK-RMSNorm.  v5: bf16 datapath, GPSIMD rank-1 offload, phased.

Sharding: 8 cores = batch(4) x head-group(2, 3 heads each). Each core:
  - phase A: Q/K/V projections, RoPE + RMSNorm on Q/K (Sqrt act-table).
    Partition-dim sums and broadcasts for the norm run on GPSIMD
    (partition_all_reduce / partition_broadcast), not the PE -- the PE only
    does real matmuls + the RoPE half-swap permutation.
  - phase B: causal attention (Exp act-table) with scores transposed
    (S^T: T_k on partitions, T_q free), heads round-robin per k-chunk to
    hide exp latency; softmax denominator broadcast on GPSIMD; c_proj after
    each T_q tile.
  - host sums the two head-group partials per batch.
All SBUF tiles bf16 (2x DVE, half DMA bytes); PSUM f32. Activation table
loads: 2 total (Sqrt set in phase A, Exp set in phase B).
"""

import numpy as np
import ml_dtypes

_B, _T, _C, _H, _D = 4, 2048, 768, 6, 128
_HPG = 3            # heads per group
_HD = _HPG * _D     # 384, per-group head dims
_NT = 4             # T tiles of 512
_TW = 512           # tile width (T_q)
_NKC = _T // 128    # 16 k-chunks of 128
_NCB = _C // 128    # 6 c_in chunks
_EPS = 1e-15

_cached = {}

# schedule knobs (overridable for TimelineSim sweeps)
_KNOBS = {
    "lookahead": 4,
    "pa_bufs": 15,
    "att0_overlap": True,
    "sprinkle": 2,
    "ppS": 3,
    "ppW": 1,
}


def _build_nc():
    from contextlib import ExitStack
    from concourse import bacc, tile, mybir, bass_isa

    f32 = mybir.dt.float32
    bf16 = mybir.dt.bfloat16
    Act = mybir.ActivationFunctionType
    Red = bass_isa.ReduceOp

    nc = bacc.Bacc("TRN2", target_bir_lowering=False, debug=False)

    xT = nc.dram_tensor("xT", (128, _NCB * _T), bf16, kind="ExternalInput").ap()
    wq = nc.dram_tensor("wq", (128, _NCB * _HD), bf16, kind="ExternalInput").ap()
    wk = nc.dram_tensor("wk", (128, _NCB * _HD), bf16, kind="ExternalInput").ap()
    wv = nc.dram_tensor("wv", (128, _NCB * _HD), bf16, kind="ExternalInput").ap()
    wo = nc.dram_tensor("wo", (128, _HPG * _C), bf16, kind="ExternalInput").ap()
    cs = nc.dram_tensor("cs", (128, 2 * _T), bf16, kind="ExternalInput").ap()
    cst = nc.dram_tensor("cst", (128, 3 * 128), bf16, kind="ExternalInput").ap()
    out = nc.dram_tensor("out", (128, _NT * 4 * _C), bf16, kind="ExternalOutput").ap()

    with tile.TileContext(nc) as tc, ExitStack() as ctx, \
            nc.allow_low_precision(reason="bf16 datapath; f32 psum accumulation"):
        # --- pools ---
        pc = ctx.enter_context(tc.tile_pool(name="pc", bufs=1))         # persistents
        pg = ctx.enter_context(tc.tile_pool(name="pg", bufs=3))         # rope/sq scratch
        pa = ctx.enter_context(tc.tile_pool(name="pa", bufs=_KNOBS["pa_bufs"]))  # A chunks
        psm = ctx.enter_context(tc.tile_pool(name="psm", bufs=3))       # small vectors
        pdb = ctx.enter_context(tc.tile_pool(name="pdb", bufs=2))       # bcast tiles
        pob = ctx.enter_context(tc.tile_pool(name="pob", bufs=2))       # out staging
        # psum pools (8 banks): proj/S x2, sw/c_proj/db x2, O x3, den x1 --
        # this keeps attention-0's accumulators alive while phase A's rope
        # still allocates sw psums.
        ppS = ctx.enter_context(tc.tile_pool(name="ppS", bufs=_KNOBS["ppS"], space="PSUM"))
        ppW = ctx.enter_context(tc.tile_pool(name="ppW", bufs=_KNOBS["ppW"], space="PSUM"))
        ppO = ctx.enter_context(tc.tile_pool(name="ppO", bufs=3, space="PSUM"))
        ppM = ctx.enter_context(tc.tile_pool(name="ppM", bufs=1, space="PSUM"))

        # --- DMA inputs (host pre-packs layouts; all bf16) ---
        t_xt = pc.tile([128, _NCB * _T], bf16, tag="xt", name="t_xt")
        t_wk = pc.tile([128, _NCB * _HD], bf16, tag="wk", name="t_wk")
        nc.sync.dma_start(t_wk[:, 0:_HD], wk[:, 0:_HD])
        nc.sync.dma_start(t_xt[:, 0:_T], xT[:, 0:_T])
        nc.sync.dma_start(t_wk[:, _HD:], wk[:, _HD:])
        for c in range(1, _NCB):
            nc.sync.dma_start(t_xt[:, c * _T:(c + 1) * _T], xT[:, c * _T:(c + 1) * _T])
        t_cs = pc.tile([128, 2 * _T], bf16, tag="cs", name="t_cs")
        nc.sync.dma_start(t_cs[:], cs[:])
        t_cst = pc.tile([128, 3 * 128], bf16, tag="cst", name="t_cst")
        nc.sync.dma_start(t_cst[:], cst[:])
        t_wv = pc.tile([128, _NCB * _HD], bf16, tag="wv", name="t_wv")
        nc.sync.dma_start(t_wv[:], wv[:])
        t_wq = pc.tile([128, _NCB * _HD], bf16, tag="wq", name="t_wq")
        nc.sync.dma_start(t_wq[:], wq[:])
        t_wo = pc.tile([128, _HPG * _C], bf16, tag="wo", name="t_wo")
        nc.sync.dma_start(t_wo[:], wo[:])
        t_eps = pc.tile([128, 1], f32, tag="eps", name="t_eps")
        nc.gpsimd.memset(t_eps[:], _EPS)

        t_tri = t_cst[:, 0:128]
        t_ones_col = t_cst[:, 128:129]
        t_perm = t_cst[:, 256:384]

        # persistent per-head K^T/Q^T (post rope+norm) and V blocks
        t_kn = [pc.tile([128, _T], bf16, tag=f"kn{h}", name=f"kn{h}") for h in range(_HPG)]
        t_qn = [pc.tile([128, _T], bf16, tag=f"qn{h}", name=f"qn{h}") for h in range(_HPG)]
        t_v = [pc.tile([128, _HD], bf16, tag=f"v{tb}", name=f"v{tb}") for tb in range(_NKC)]

        # ---------------- phase A emitters ----------------
        def proj_qk(dst, w, i, h):
            """dst[:, i*512:(i+1)*512] = (W x)^T tile for head h."""
            p = ppS.tile([128, _TW], f32, tag="pS", name="p_qk")
            for c in range(_NCB):
                nc.tensor.matmul(
                    p[:], w[:, c * _HD + h * 128: c * _HD + (h + 1) * 128],
                    t_xt[:, c * _T + i * _TW: c * _T + (i + 1) * _TW],
                    start=(c == 0), stop=(c == _NCB - 1))
            nc.scalar.copy(dst[:, i * _TW:(i + 1) * _TW], p[:])

        def proj_v(tb):
            p = ppS.tile([128, _HD], f32, tag="pS", name="p_v",
                         padded_shape=[128, _TW])
            for c in range(_NCB):
                nc.tensor.matmul(
                    p[:], t_xt[:, c * _T + tb * 128: c * _T + (tb + 1) * 128],
                    t_wv[:, c * _HD:(c + 1) * _HD],
                    start=(c == 0), stop=(c == _NCB - 1))
            nc.scalar.copy(t_v[tb][:], p[:])

        def rope_stage(dst_list, i):
            """In-place RoPE on dst[h][:, i*512:+512] for all heads."""
            isl = slice(i * _TW, (i + 1) * _TW)
            ssl = slice(_T + i * _TW, _T + (i + 1) * _TW)
            sws = []
            for h in range(_HPG):
                p_sw = ppW.tile([128, _TW], f32, tag="pW", name="p_sw")
                nc.tensor.matmul(p_sw[:], t_perm, dst_list[h][:, isl],
                                 start=True, stop=True)
                t_sw = pg.tile([128, _TW], bf16, tag="sw", name="t_sw", bufs=3)
                nc.vector.tensor_mul(t_sw[:], p_sw[:], t_cs[:, ssl])
                sws.append(t_sw)
            for h in range(_HPG):
                nc.vector.tensor_mul(dst_list[h][:, isl], dst_list[h][:, isl],
                                     t_cs[:, isl])
                nc.vector.tensor_add(dst_list[h][:, isl], dst_list[h][:, isl],
                                     sws[h][:])

        def norm_stage(dst_list, i):
            """In-place RMSNorm over partitions (D): square (DVE) ->
            partition all-reduce (GPSIMD) -> sqrt on a (1,512) row (Act) ->
            reciprocal (DVE) -> broadcast (GPSIMD) -> scale (DVE, all-SBUF
            bf16 so it runs in 2x mode)."""
            isl = slice(i * _TW, (i + 1) * _TW)
            sums, bcs = [], []
            for h in range(_HPG):
                t_sq = pg.tile([128, _TW], bf16, tag="sq", name="t_sq", bufs=3)
                nc.vector.tensor_mul(t_sq[:], dst_list[h][:, isl],
                                     dst_list[h][:, isl])
                t_ms = pg.tile([128, _TW], bf16, tag="ms", name="t_ms", bufs=3)
                nc.gpsimd.partition_all_reduce(t_ms[:], t_sq[:], 128, Red.add)
                sums.append(t_ms)
            for h in range(_HPG):
                t_sd = psm.tile([1, _TW], bf16, tag="sd", name="t_sd", bufs=3)
                nc.scalar.activation(t_sd[:], sums[h][0:1, :], Act.Sqrt,
                                     bias=t_eps[0:1, :], scale=1.0 / 128.0)
                t_rs = psm.tile([1, _TW], bf16, tag="rs", name="t_rs", bufs=3)
                nc.vector.reciprocal(t_rs[:], t_sd[:])
                t_bc = pdb.tile([128, _TW], bf16, tag="bc", name="t_bc")
                nc.gpsimd.partition_broadcast(t_bc[:], t_rs[:])
                bcs.append(t_bc)
            for h in range(_HPG):
                nc.vector.tensor_mul(dst_list[h][:, isl], dst_list[h][:, isl],
                                     bcs[h][:])

        # ---------------- phase B ----------------
        def attention_rounds(qt):
            """Causal attention for all 3 heads of one T_q tile, heads
            round-robin per k-chunk. Returns (round_closures, finisher);
            the finisher returns the z tiles."""
            nchunk = 4 * qt + 4
            LOOKAHEAD = _KNOBS["lookahead"]
            st = {}

            def init():
                st["den"] = ppM.tile([65, _TW], f32, tag="pM", name="p_den")
                st["os"] = [ppO.tile([128, _TW], f32, tag="pO", name=f"p_o{h}")
                            for h in range(_HPG)]
                st["a"] = {}

            def emit_s(kc, h):
                roff = 0 if kc < 4 * qt else (kc - 4 * qt) * 128
                nsl = slice(roff, _TW)
                ksl = slice(kc * 128, (kc + 1) * 128)
                p_s = ppS.tile([128, _TW], f32, tag="pS", name="p_s")
                nc.tensor.matmul(p_s[:, nsl], t_kn[h][:, ksl],
                                 t_qn[h][:, qt * _TW + roff:(qt + 1) * _TW],
                                 start=True, stop=True)
                t_a = pa.tile([128, _TW], bf16, tag="a", name="t_a")
                nc.scalar.activation(t_a[:, nsl], p_s[:, nsl], Act.Exp,
                                     scale=1.0 / float(np.sqrt(_D)))
                if kc >= 4 * qt:  # diagonal chunk: triangular mask
                    dsl = slice(roff, roff + 128)
                    nc.gpsimd.tensor_mul(t_a[:, dsl], t_a[:, dsl], t_tri)
                st["a"][(kc, h)] = t_a

            def emit_acc(kc, h):
                roff = 0 if kc < 4 * qt else (kc - 4 * qt) * 128
                nsl = slice(roff, _TW)
                t_a = st["a"].pop((kc, h))
                nc.tensor.matmul(st["den"][32 * h:32 * h + 1, nsl], t_ones_col,
                                 t_a[:, nsl],
                                 start=(kc == 0), stop=(kc == nchunk - 1))
                nc.tensor.matmul(st["os"][h][:, nsl],
                                 t_v[kc][:, h * 128:(h + 1) * 128], t_a[:, nsl],
                                 start=(kc == 0), stop=(kc == nchunk - 1))

            def make_round(kc):
                def r():
                    if kc == 0:
                        init()
                    for h in range(_HPG):
                        if kc < nchunk:
                            emit_s(kc, h)
                        if kc >= LOOKAHEAD:
                            emit_acc(kc - LOOKAHEAD, h)
                return r

            def finisher():
                # extract dens, PE broadcast (partition_broadcast cannot read
                # partition-offset rows), reciprocal, mul
                t_dn = psm.tile([65, _TW], bf16, tag="dn", name="t_dn", bufs=3)
                nc.scalar.copy(t_dn[:], st["den"][:])
                zs = []
                for h in range(_HPG):
                    p_db = ppW.tile([128, _TW], f32, tag="pW", name="p_db")
                    nc.tensor.matmul(p_db[:], t_cst[32 * h:32 * h + 1, 128:256],
                                     t_dn[32 * h:32 * h + 1, :],
                                     start=True, stop=True)
                    t_db = pdb.tile([128, _TW], bf16, tag="db", name="t_db")
                    nc.vector.reciprocal(t_db[:], p_db[:])
                    t_z = pc.tile([128, _TW], bf16, tag=f"z{h}_{qt % 2}",
                                  name=f"z{h}_{qt % 2}")
                    nc.vector.tensor_mul(t_z[:], st["os"][h][:], t_db[:])
                    zs.append(t_z)
                return zs

            rounds = [make_round(kc) for kc in range(nchunk + _KNOBS["lookahead"])]
            return rounds, finisher

        def attention(qt):
            rounds, fin = attention_rounds(qt)
            for r in rounds:
                r()
            return fin()

        _ob = {}

        def c_proj_half(qt, zs, blks):
            if qt not in _ob:
                _ob[qt] = pob.tile([128, 4 * _C], bf16, tag="ob", name="t_ob")
            t_ob = _ob[qt]
            for blk in blks:
                bsl = slice(blk * 128, (blk + 1) * 128)
                for nh in range(2):
                    p_c = ppW.tile([128, 384], f32, tag="pW", name="p_c",
                                   padded_shape=[128, _TW])
                    for hh in range(_HPG):
                        nc.tensor.matmul(
                            p_c[:], zs[hh][:, bsl],
                            t_wo[:, hh * _C + nh * 384: hh * _C + (nh + 1) * 384],
                            start=(hh == 0), stop=(hh == _HPG - 1))
                    o0 = blk * _C + nh * 384
                    nc.vector.tensor_copy(t_ob[:, o0:o0 + 384], p_c[:])
            if 3 in blks:
                nc.sync.dma_start(out[:, qt * 4 * _C:(qt + 1) * 4 * _C], t_ob[:])

        # ---------------- emission schedule ----------------
        # Phase A: K projs (DMA-paced), then per tile rope/norm of K and Q
        # with a fill queue of independent PE work in the dependency windows:
        # V blocks 0..7 first, then attention(0)'s chunk rounds (it only
        # needs K/Q tile 0 + V blocks 0..3). V blocks 8..15 are deferred to
        # phase B (their extract is Copy = Exp-table, no act-table switch)
        # where they fill the qt=1 pipeline windup.
        for i in range(_NT):
            for h in range(_HPG):
                proj_qk(t_kn[h], t_wk, i, h)
        fillq = [lambda tb=tb: (proj_v(2 * tb), proj_v(2 * tb + 1))
                 for tb in range(8)]             # V pairs 0..15
        fin0 = None

        def fill(n):
            for _ in range(n):
                if fillq:
                    fillq.pop(0)()

        for i in range(_NT):
            rope_stage(t_kn, i)
            fill(2)
            norm_stage(t_kn, i)
            for h in range(_HPG):                # PE filler while K norm runs
                proj_qk(t_qn[h], t_wq, i, h)
            fill(1)
            rope_stage(t_qn, i)
            fill(1)
            norm_stage(t_qn, i)
            if i == 0:
                rounds0, fin0 = attention_rounds(0)
                if _KNOBS["att0_overlap"]:
                    fillq.extend(rounds0)
                else:
                    deferred0 = list(rounds0)
            else:
                fill(1)
        fill(len(fillq))
        if not _KNOBS["att0_overlap"]:
            for r in deferred0:
                r()
        prev = (0, fin0())
        for qt in range(1, _NT):
            rounds, fin = attention_rounds(qt)
            pqt, pzs = prev
            c_proj_half(pqt, pzs, [0, 1])
            c_proj_half(pqt, pzs, [2, 3])
            for r in rounds:
                r()
            prev = (qt, fin())
        qt, zs = prev
        c_proj_half(qt, zs, [0, 1])
        c_proj_half(qt, zs, [2, 3])

    nc.compile()
    return nc


def _get_nc():
    if "nc" not in _cached:
        _cached["nc"] = _build_nc()
    return _cached["nc"]


def _bf16(a):
    return np.ascontiguousarray(a.astype(ml_dtypes.bfloat16))


def make_in_maps(x, cos, sin, Wq, Wk, Wv, Wo):
    cosT = cos.reshape(_T, _D // 2).T                        # (64, T)
    sinT = sin.reshape(_T, _D // 2).T
    cc = np.concatenate([cosT, cosT], axis=0)                # (128, T)
    ss = np.concatenate([sinT, -sinT], axis=0)
    cs = _bf16(np.concatenate([cc, ss], axis=1))             # (128, 2T)
    tri = (np.arange(128)[None, :] >= np.arange(128)[:, None]).astype(np.float32)
    ones128 = np.ones((128, 128), dtype=np.float32)
    permm = np.zeros((128, 128), dtype=np.float32)           # half-swap permutation
    for d in range(64):
        permm[64 + d, d] = 1.0
        permm[d, 64 + d] = 1.0
    cst = _bf16(np.concatenate([tri, ones128, permm], axis=1))

    def pack_w(w):  # (768, 384) -> (128, 2304) c-chunk-major
        return _bf16(w.reshape(_NCB, 128, _HD).transpose(1, 0, 2).reshape(128, -1))

    def pack_wo(w):  # (384, 768) -> (128, 2304) head-chunk-major
        return _bf16(w.reshape(_HPG, 128, _C).transpose(1, 0, 2).reshape(128, -1))

    in_maps = []
    for core in range(8):
        b, g = divmod(core, 2)
        gsl = slice(g * _HD, (g + 1) * _HD)
        xt = x[b].T.reshape(_NCB, 128, _T).transpose(1, 0, 2).reshape(128, -1)
        in_maps.append({
            "xT": _bf16(xt),
            "wq": pack_w(Wq[gsl, :].T),
            "wk": pack_w(Wk[gsl, :].T),
            "wv": pack_w(Wv[gsl, :].T),
            "wo": pack_wo(Wo[:, gsl].T),
            "cs": cs, "cst": cst,
        })
    return in_maps


def unshard(core_outs):
    """core_outs: list of 8 per-core (128, 12288) bf16 arrays -> (4, 2048, 768)."""
    full = []
    for b in range(_B):
        acc = None
        for g in range(2):
            dev = np.asarray(core_outs[2 * b + g]).astype(np.float32)
            part = dev.reshape(128, _NT, 4, _C).transpose(1, 2, 0, 3).reshape(_T, _C)
            acc = part if acc is None else acc + part
        full.append(acc)
    return np.stack(full, axis=0)


def kernel(x, cos, sin, Wq, Wk, Wv, Wo):
    from concourse.bass_utils import run_bass_kernel_spmd

    x = np.asarray(x, dtype=np.float32)
    cos = np.asarray(cos, dtype=np.float32)
    sin = np.asarray(sin, dtype=np.float32)
    Wq = np.asarray(Wq, dtype=np.float32)
    Wk = np.asarray(Wk, dtype=np.float32)
    Wv = np.asarray(Wv, dtype=np.float32)
    Wo = np.asarray(Wo, dtype=np.float32)

    nc = _get_nc()
    in_maps = make_in_maps(x, cos, sin, Wq, Wk, Wv, Wo)
    res = run_bass_kernel_spmd(nc, in_maps, core_ids=list(range(8)))
    return unshard([r_["out"] for r_ in res.results])


# revision 60
# speedup vs baseline: 1.0663x; 1.0173x over previous
"""Trainium2 Bass kernel for CausalSelfAttention (B=4, T=2048, C=768, H=6, D=128)
with RoPE + QK-RMSNorm.  v5: bf16 datapath, GPSIMD rank-1 offload, phased.

Sharding: 8 cores = batch(4) x head-group(2, 3 heads each). Each core:
  - phase A: Q/K/V projections, RoPE + RMSNorm on Q/K (Sqrt act-table).
    Partition-dim sums and broadcasts for the norm run on GPSIMD
    (partition_all_reduce / partition_broadcast), not the PE -- the PE only
    does real matmuls + the RoPE half-swap permutation.
  - phase B: causal attention (Exp act-table) with scores transposed
    (S^T: T_k on partitions, T_q free), heads round-robin per k-chunk to
    hide exp latency; softmax denominator broadcast on GPSIMD; c_proj after
    each T_q tile.
  - host sums the two head-group partials per batch.
All SBUF tiles bf16 (2x DVE, half DMA bytes); PSUM f32. Activation table
loads: 2 total (Sqrt set in phase A, Exp set in phase B).
"""

import numpy as np
import ml_dtypes

_B, _T, _C, _H, _D = 4, 2048, 768, 6, 128
_HPG = 3            # heads per group
_HD = _HPG * _D     # 384, per-group head dims
_NT = 4             # T tiles of 512
_TW = 512           # tile width (T_q)
_NKC = _T // 128    # 16 k-chunks of 128
_NCB = _C // 128    # 6 c_in chunks
_EPS = 1e-15

_cached = {}

# schedule knobs (overridable for TimelineSim sweeps)
_KNOBS = {
    "lookahead": 4,
    "pa_bufs": 15,
    "att0_overlap": True,
    "sprinkle": 2,
    "ppS": 3,
    "ppW": 1,
}


def _build_nc():
    from contextlib import ExitStack
    from concourse import bacc, tile, mybir, bass_isa

    f32 = mybir.dt.float32
    bf16 = mybir.dt.bfloat16
    Act = mybir.ActivationFunctionType
    Red = bass_isa.ReduceOp

    nc = bacc.Bacc("TRN2", target_bir_lowering=False, debug=False)

    xT = nc.dram_tensor("xT", (128, _NCB * _T), bf16, kind="ExternalInput").ap()
    wq = nc.dram_tensor("wq", (128, _NCB * _HD), bf16, kind="ExternalInput").ap()
    wk = nc.dram_tensor("wk", (128, _NCB * _HD), bf16, kind="ExternalInput").ap()
    wv = nc.dram_tensor("wv", (128, _NCB * _HD), bf16, kind="ExternalInput").ap()
    wo = nc.dram_tensor("wo", (128, _HPG * _C), bf16, kind="ExternalInput").ap()
    cs = nc.dram_tensor("cs", (128, 2 * _T), bf16, kind="ExternalInput").ap()
    cst = nc.dram_tensor("cst", (128, 3 * 128), bf16, kind="ExternalInput").ap()
    out = nc.dram_tensor("out", (128, _NT * 4 * _C), bf16, kind="ExternalOutput").ap()

    with tile.TileContext(nc) as tc, ExitStack() as ctx, \
            nc.allow_low_precision(reason="bf16 datapath; f32 psum accumulation"):
        # --- pools ---
        pc = ctx.enter_context(tc.tile_pool(name="pc", bufs=1))         # persistents
        pg = ctx.enter_context(tc.tile_pool(name="pg", bufs=3))         # rope/sq scratch
        pa = ctx.enter_context(tc.tile_pool(name="pa", bufs=_KNOBS["pa_bufs"]))  # A chunks
        psm = ctx.enter_context(tc.tile_pool(name="psm", bufs=3))       # small vectors
        pdb = ctx.enter_context(tc.tile_pool(name="pdb", bufs=2))       # bcast tiles
        pob = ctx.enter_context(tc.tile_pool(name="pob", bufs=2))       # out staging
        # psum pools (8 banks): proj/S x2, sw/c_proj/db x2, O x3, den x1 --
        # this keeps attention-0's accumulators alive while phase A's rope
        # still allocates sw psums.
        ppS = ctx.enter_context(tc.tile_pool(name="ppS", bufs=_KNOBS["ppS"], space="PSUM"))
        ppW = ctx.enter_context(tc.tile_pool(name="ppW", bufs=_KNOBS["ppW"], space="PSUM"))
        ppO = ctx.enter_context(tc.tile_pool(name="ppO", bufs=3, space="PSUM"))
        ppM = ctx.enter_context(tc.tile_pool(name="ppM", bufs=1, space="PSUM"))

        # --- DMA inputs (host pre-packs layouts; all bf16) ---
        # x arrives as 12 half-chunk pieces, T-half-major (host packs DRAM in
        # this exact order): all 6 c-chunks of tiles 0-1 land first (~4.4us)
        # so the tile-0/1 K-projections start early.
        t_xt = pc.tile([128, _NCB * _T], bf16, tag="xt", name="t_xt")
        t_wk = pc.tile([128, _NCB * _HD], bf16, tag="wk", name="t_wk")
        nc.sync.dma_start(t_wk[:], wk[:])
        for j in range(2):
            for c in range(_NCB):
                nc.sync.dma_start(
                    t_xt[:, c * _T + j * 1024: c * _T + (j + 1) * 1024],
                    xT[:, (j * _NCB + c) * 1024: (j * _NCB + c + 1) * 1024])
        t_cs = pc.tile([128, 2 * _T], bf16, tag="cs", name="t_cs")
        nc.sync.dma_start(t_cs[:], cs[:])
        t_cst = pc.tile([128, 3 * 128], bf16, tag="cst", name="t_cst")
        nc.sync.dma_start(t_cst[:], cst[:])
        t_wv = pc.tile([128, _NCB * _HD], bf16, tag="wv", name="t_wv")
        nc.sync.dma_start(t_wv[:], wv[:])
        t_wq = pc.tile([128, _NCB * _HD], bf16, tag="wq", name="t_wq")
        nc.sync.dma_start(t_wq[:], wq[:])
        t_wo = pc.tile([128, _HPG * _C], bf16, tag="wo", name="t_wo")
        nc.sync.dma_start(t_wo[:], wo[:])
        t_eps = pc.tile([128, 1], f32, tag="eps", name="t_eps")
        nc.gpsimd.memset(t_eps[:], _EPS)

        t_tri = t_cst[:, 0:128]
        t_ones_col = t_cst[:, 128:129]
        t_perm = t_cst[:, 256:384]

        # persistent per-head K^T/Q^T (post rope+norm) and V blocks
        t_kn = [pc.tile([128, _T], bf16, tag=f"kn{h}", name=f"kn{h}") for h in range(_HPG)]
        t_qn = [pc.tile([128, _T], bf16, tag=f"qn{h}", name=f"qn{h}") for h in range(_HPG)]
        t_v = [pc.tile([128, _HD], bf16, tag=f"v{tb}", name=f"v{tb}") for tb in range(_NKC)]

        # ---------------- phase A emitters ----------------
        def proj_qk(dst, w, i, h):
            """dst[:, i*512:(i+1)*512] = (W x)^T tile for head h."""
            p = ppS.tile([128, _TW], f32, tag="pS", name="p_qk")
            for c in range(_NCB):
                nc.tensor.matmul(
                    p[:], w[:, c * _HD + h * 128: c * _HD + (h + 1) * 128],
                    t_xt[:, c * _T + i * _TW: c * _T + (i + 1) * _TW],
                    start=(c == 0), stop=(c == _NCB - 1))
            nc.scalar.copy(dst[:, i * _TW:(i + 1) * _TW], p[:])

        def proj_v(tb):
            p = ppS.tile([128, _HD], f32, tag="pS", name="p_v",
                         padded_shape=[128, _TW])
            for c in range(_NCB):
                nc.tensor.matmul(
                    p[:], t_xt[:, c * _T + tb * 128: c * _T + (tb + 1) * 128],
                    t_wv[:, c * _HD:(c + 1) * _HD],
                    start=(c == 0), stop=(c == _NCB - 1))
            nc.scalar.copy(t_v[tb][:], p[:])

        def rope_stage(dst_list, i):
            """In-place RoPE on dst[h][:, i*512:+512] for all heads."""
            isl = slice(i * _TW, (i + 1) * _TW)
            ssl = slice(_T + i * _TW, _T + (i + 1) * _TW)
            sws = []
            for h in range(_HPG):
                p_sw = ppW.tile([128, _TW], f32, tag="pW", name="p_sw")
                nc.tensor.matmul(p_sw[:], t_perm, dst_list[h][:, isl],
                                 start=True, stop=True)
                t_sw = pg.tile([128, _TW], bf16, tag="sw", name="t_sw", bufs=3)
                nc.vector.tensor_mul(t_sw[:], p_sw[:], t_cs[:, ssl])
                sws.append(t_sw)
            for h in range(_HPG):
                nc.vector.tensor_mul(dst_list[h][:, isl], dst_list[h][:, isl],
                                     t_cs[:, isl])
                nc.vector.tensor_add(dst_list[h][:, isl], dst_list[h][:, isl],
                                     sws[h][:])

        def norm_stage(dst_list, i):
            """In-place RMSNorm over partitions (D): square (DVE) ->
            partition all-reduce (GPSIMD) -> sqrt on a (1,512) row (Act) ->
            reciprocal (DVE) -> broadcast (GPSIMD) -> scale (DVE, all-SBUF
            bf16 so it runs in 2x mode)."""
            isl = slice(i * _TW, (i + 1) * _TW)
            sums, bcs = [], []
            for h in range(_HPG):
                t_sq = pg.tile([128, _TW], bf16, tag="sq", name="t_sq", bufs=3)
                nc.vector.tensor_mul(t_sq[:], dst_list[h][:, isl],
                                     dst_list[h][:, isl])
                t_ms = pg.tile([128, _TW], bf16, tag="ms", name="t_ms", bufs=3)
                nc.gpsimd.partition_all_reduce(t_ms[:], t_sq[:], 128, Red.add)
                sums.append(t_ms)
            for h in range(_HPG):
                t_sd = psm.tile([1, _TW], bf16, tag="sd", name="t_sd", bufs=3)
                nc.scalar.activation(t_sd[:], sums[h][0:1, :], Act.Sqrt,
                                     bias=t_eps[0:1, :], scale=1.0 / 128.0)
                t_rs = psm.tile([1, _TW], bf16, tag="rs", name="t_rs", bufs=3)
                nc.vector.reciprocal(t_rs[:], t_sd[:])
                t_bc = pdb.tile([128, _TW], bf16, tag="bc", name="t_bc")
                nc.gpsimd.partition_broadcast(t_bc[:], t_rs[:])
                bcs.append(t_bc)
            for h in range(_HPG):
                nc.vector.tensor_mul(dst_list[h][:, isl], dst_list[h][:, isl],
                                     bcs[h][:])

        # ---------------- phase B ----------------
        def attention_rounds(qt, split_acc=False):
            """Causal attention for all 3 heads of one T_q tile, heads
            round-robin per k-chunk. Returns (round_closures, finisher);
            the finisher returns the z tiles."""
            nchunk = 4 * qt + 4
            LOOKAHEAD = _KNOBS["lookahead"]
            st = {}

            st["a"] = {}

            def init():
                st["den"] = ppM.tile([65, _TW], f32, tag="pM", name="p_den")
                st["os"] = [ppO.tile([128, _TW], f32, tag="pO", name=f"p_o{h}")
                            for h in range(_HPG)]

            def emit_s(kc, h):
                roff = 0 if kc < 4 * qt else (kc - 4 * qt) * 128
                nsl = slice(roff, _TW)
                ksl = slice(kc * 128, (kc + 1) * 128)
                p_s = ppS.tile([128, _TW], f32, tag="pS", name="p_s")
                nc.tensor.matmul(p_s[:, nsl], t_kn[h][:, ksl],
                                 t_qn[h][:, qt * _TW + roff:(qt + 1) * _TW],
                                 start=True, stop=True)
                t_a = pa.tile([128, _TW], bf16, tag="a", name="t_a")
                nc.scalar.activation(t_a[:, nsl], p_s[:, nsl], Act.Exp,
                                     scale=1.0 / float(np.sqrt(_D)))
                if kc >= 4 * qt:  # diagonal chunk: triangular mask
                    dsl = slice(roff, roff + 128)
                    nc.gpsimd.tensor_mul(t_a[:, dsl], t_a[:, dsl], t_tri)
                st["a"][(kc, h)] = t_a

            def emit_acc(kc, h):
                roff = 0 if kc < 4 * qt else (kc - 4 * qt) * 128
                nsl = slice(roff, _TW)
                t_a = st["a"].pop((kc, h))
                nc.tensor.matmul(st["den"][32 * h:32 * h + 1, nsl], t_ones_col,
                                 t_a[:, nsl],
                                 start=(kc == 0), stop=(kc == nchunk - 1))
                nc.tensor.matmul(st["os"][h][:, nsl],
                                 t_v[kc][:, h * 128:(h + 1) * 128], t_a[:, nsl],
                                 start=(kc == 0), stop=(kc == nchunk - 1))

            def make_round(kc):
                def r():
                    if kc == 0:
                        init()
                    for h in range(_HPG):
                        if kc < nchunk:
                            emit_s(kc, h)
                        if kc >= LOOKAHEAD:
                            emit_acc(kc - LOOKAHEAD, h)
                return r

            def make_s_round(kc):
                def r():
                    for h in range(_HPG):
                        emit_s(kc, h)
                return r

            def make_acc_round(kc):
                def r():
                    if kc == 0:
                        init()
                    for h in range(_HPG):
                        emit_acc(kc, h)
                return r

            def finisher():
                # extract dens, PE broadcast (partition_broadcast cannot read
                # partition-offset rows), reciprocal, mul
                t_dn = psm.tile([65, _TW], bf16, tag="dn", name="t_dn", bufs=3)
                nc.scalar.copy(t_dn[:], st["den"][:])
                zs = []
                for h in range(_HPG):
                    p_db = ppW.tile([128, _TW], f32, tag="pW", name="p_db")
                    nc.tensor.matmul(p_db[:], t_cst[32 * h:32 * h + 1, 128:256],
                                     t_dn[32 * h:32 * h + 1, :],
                                     start=True, stop=True)
                    t_db = pdb.tile([128, _TW], bf16, tag="db", name="t_db")
                    nc.vector.reciprocal(t_db[:], p_db[:])
                    t_z = pc.tile([128, _TW], bf16, tag=f"z{h}_{qt % 2}",
                                  name=f"z{h}_{qt % 2}")
                    nc.vector.tensor_mul(t_z[:], st["os"][h][:], t_db[:])
                    zs.append(t_z)
                return zs

            if split_acc:
                # S/exp rounds separated from pure-PE den/AV rounds: the
                # S rounds ride along phase A; the accs fill phase B windup.
                s_rounds = [make_s_round(kc) for kc in range(nchunk)]
                acc_rounds = [make_acc_round(kc) for kc in range(nchunk)]
                return s_rounds, acc_rounds, finisher
            rounds = [make_round(kc) for kc in range(nchunk + _KNOBS["lookahead"])]
            return rounds, finisher

        def attention(qt):
            rounds, fin = attention_rounds(qt)
            for r in rounds:
                r()
            return fin()

        _ob = {}

        def c_proj_half(qt, zs, blks):
            if qt not in _ob:
                _ob[qt] = pob.tile([128, 4 * _C], bf16, tag="ob", name="t_ob")
            t_ob = _ob[qt]
            for blk in blks:
                bsl = slice(blk * 128, (blk + 1) * 128)
                for nh in range(2):
                    p_c = ppW.tile([128, 384], f32, tag="pW", name="p_c",
                                   padded_shape=[128, _TW])
                    for hh in range(_HPG):
                        nc.tensor.matmul(
                            p_c[:], zs[hh][:, bsl],
                            t_wo[:, hh * _C + nh * 384: hh * _C + (nh + 1) * 384],
                            start=(hh == 0), stop=(hh == _HPG - 1))
                    o0 = blk * _C + nh * 384
                    nc.vector.tensor_copy(t_ob[:, o0:o0 + 384], p_c[:])
            if 3 in blks:
                nc.sync.dma_start(out[:, qt * 4 * _C:(qt + 1) * 4 * _C], t_ob[:])

        # ---------------- emission schedule ----------------
        # Phase A: K projs (DMA-paced), then per tile rope/norm of K and Q
        # with a fill queue of independent PE work in the dependency windows:
        # V blocks 0..7 first, then attention(0)'s chunk rounds (it only
        # needs K/Q tile 0 + V blocks 0..3). V blocks 8..15 are deferred to
        # phase B (their extract is Copy = Exp-table, no act-table switch)
        # where they fill the qt=1 pipeline windup.
        for i in range(_NT):
            for h in range(_HPG):
                proj_qk(t_kn[h], t_wk, i, h)
        fillq = [lambda tb=tb: (proj_v(2 * tb), proj_v(2 * tb + 1))
                 for tb in range(8)]             # V pairs 0..15
        fin0 = None

        def fill(n):
            for _ in range(n):
                if fillq:
                    fillq.pop(0)()

        att1 = _KNOBS.get("att1_overlap", False)
        acc1 = fin1 = None
        for i in range(_NT):
            rope_stage(t_kn, i)
            fill(2)
            norm_stage(t_kn, i)
            for h in range(_HPG):                # PE filler while K norm runs
                proj_qk(t_qn[h], t_wq, i, h)
            fill(1)
            rope_stage(t_qn, i)
            fill(1)
            norm_stage(t_qn, i)
            if i == 0:
                rounds0, fin0 = attention_rounds(0)
                if _KNOBS["att0_overlap"]:
                    fillq.extend(rounds0)
                else:
                    deferred0 = list(rounds0)
            else:
                fill(1)
            if i == 1 and att1:
                s1, acc1, fin1 = attention_rounds(1, split_acc=True)
                fillq.extend(s1)
        fill(len(fillq))
        if not _KNOBS["att0_overlap"]:
            for r in deferred0:
                r()
        prev = (0, fin0())
        for qt in range(1, _NT):
            pqt, pzs = prev
            if qt == 1 and att1:
                # att1's S/exp already ran in phase A; its den/AV rounds are
                # pure PE and fill the windup here.
                for r in acc1:
                    r()
                c_proj_half(pqt, pzs, [0, 1])
                c_proj_half(pqt, pzs, [2, 3])
                prev = (1, fin1())
                continue
            rounds, fin = attention_rounds(qt)
            c_proj_half(pqt, pzs, [0, 1])
            c_proj_half(pqt, pzs, [2, 3])
            for r in rounds:
                r()
            prev = (qt, fin())
        qt, zs = prev
        c_proj_half(qt, zs, [0, 1])
        c_proj_half(qt, zs, [2, 3])

    nc.compile()
    return nc


def _get_nc():
    if "nc" not in _cached:
        _cached["nc"] = _build_nc()
    return _cached["nc"]


def _bf16(a):
    return np.ascontiguousarray(a.astype(ml_dtypes.bfloat16))


def make_in_maps(x, cos, sin, Wq, Wk, Wv, Wo):
    cosT = cos.reshape(_T, _D // 2).T                        # (64, T)
    sinT = sin.reshape(_T, _D // 2).T
    cc = np.concatenate([cosT, cosT], axis=0)                # (128, T)
    ss = np.concatenate([sinT, -sinT], axis=0)
    cs = _bf16(np.concatenate([cc, ss], axis=1))             # (128, 2T)
    tri = (np.arange(128)[None, :] >= np.arange(128)[:, None]).astype(np.float32)
    ones128 = np.ones((128, 128), dtype=np.float32)
    permm = np.zeros((128, 128), dtype=np.float32)           # half-swap permutation
    for d in range(64):
        permm[64 + d, d] = 1.0
        permm[d, 64 + d] = 1.0
    cst = _bf16(np.concatenate([tri, ones128, permm], axis=1))

    def pack_w(w):  # (768, 384) -> (128, 2304) c-chunk-major
        return _bf16(w.reshape(_NCB, 128, _HD).transpose(1, 0, 2).reshape(128, -1))

    def pack_wo(w):  # (384, 768) -> (128, 2304) head-chunk-major
        return _bf16(w.reshape(_HPG, 128, _C).transpose(1, 0, 2).reshape(128, -1))

    in_maps = []
    for core in range(8):
        b, g = divmod(core, 2)
        gsl = slice(g * _HD, (g + 1) * _HD)
        # (128, 12288) in DMA piece order: T-half-major, then c-chunk
        xt = (x[b].T.reshape(_NCB, 128, 2, 1024).transpose(1, 2, 0, 3)
              .reshape(128, -1))
        in_maps.append({
            "xT": _bf16(xt),
            "wq": pack_w(Wq[gsl, :].T),
            "wk": pack_w(Wk[gsl, :].T),
            "wv": pack_w(Wv[gsl, :].T),
            "wo": pack_wo(Wo[:, gsl].T),
            "cs": cs, "cst": cst,
        })
    return in_maps


def unshard(core_outs):
    """core_outs: list of 8 per-core (128, 12288) bf16 arrays -> (4, 2048, 768)."""
    full = []
    for b in range(_B):
        acc = None
        for g in range(2):
            dev = np.asarray(core_outs[2 * b + g]).astype(np.float32)
            part = dev.reshape(128, _NT, 4, _C).transpose(1, 2, 0, 3).reshape(_T, _C)
            acc = part if acc is None else acc + part
        full.append(acc)
    return np.stack(full, axis=0)


def kernel(x, cos, sin, Wq, Wk, Wv, Wo):
    from concourse.bass_utils import run_bass_kernel_spmd

    x = np.asarray(x, dtype=np.float32)
    cos = np.asarray(cos, dtype=np.float32)
    sin = np.asarray(sin, dtype=np.float32)
    Wq = np.asarray(Wq, dtype=np.float32)
    Wk = np.asarray(Wk, dtype=np.float32)
    Wv = np.asarray(Wv, dtype=np.float32)
    Wo = np.asarray(Wo, dtype=np.float32)

    nc = _get_nc()
    in_maps = make_in_maps(x, cos, sin, Wq, Wk, Wv, Wo)
    res = run_bass_kernel_spmd(nc, in_maps, core_ids=list(range(8)))
    return unshard([r_["out"] for r_ in res.results])


# revision 61
# speedup vs baseline: 1.0727x; 1.0060x over previous
"""Trainium2 Bass kernel for CausalSelfAttention (B=4, T=2048, C=768, H=6, D=128)
with RoPE + QK-RMSNorm.  v5: bf16 datapath, GPSIMD rank-1 offload, phased.

Sharding: 8 cores = batch(4) x head-group(2, 3 heads each). Each core:
  - phase A: Q/K/V projections, RoPE + RMSNorm on Q/K (Sqrt act-table).
    Partition-dim sums and broadcasts for the norm run on GPSIMD
    (partition_all_reduce / partition_broadcast), not the PE -- the PE only
    does real matmuls + the RoPE half-swap permutation.
  - phase B: causal attention (Exp act-table) with scores transposed
    (S^T: T_k on partitions, T_q free), heads round-robin per k-chunk to
    hide exp latency; softmax denominator broadcast on GPSIMD; c_proj after
    each T_q tile.
  - host sums the two head-group partials per batch.
All SBUF tiles bf16 (2x DVE, half DMA bytes); PSUM f32. Activation table
loads: 2 total (Sqrt set in phase A, Exp set in phase B).
"""

import numpy as np
import ml_dtypes

_B, _T, _C, _H, _D = 4, 2048, 768, 6, 128
_HPG = 3            # heads per group
_HD = _HPG * _D     # 384, per-group head dims
_NT = 4             # T tiles of 512
_TW = 512           # tile width (T_q)
_NKC = _T // 128    # 16 k-chunks of 128
_NCB = _C // 128    # 6 c_in chunks
_EPS = 1e-15

_cached = {}

# schedule knobs (overridable for TimelineSim sweeps)
_KNOBS = {
    "lookahead": 3,
    "pa_bufs": 40,
    "att0_overlap": True,
    "att1_overlap": True,
    "sprinkle": 2,
    "ppS": 3,
    "ppW": 1,
}


def _build_nc():
    from contextlib import ExitStack
    from concourse import bacc, tile, mybir, bass_isa

    f32 = mybir.dt.float32
    bf16 = mybir.dt.bfloat16
    Act = mybir.ActivationFunctionType
    Red = bass_isa.ReduceOp

    nc = bacc.Bacc("TRN2", target_bir_lowering=False, debug=False)

    xT = nc.dram_tensor("xT", (128, _NCB * _T), bf16, kind="ExternalInput").ap()
    wq = nc.dram_tensor("wq", (128, _NCB * _HD), bf16, kind="ExternalInput").ap()
    wk = nc.dram_tensor("wk", (128, _NCB * _HD), bf16, kind="ExternalInput").ap()
    wv = nc.dram_tensor("wv", (128, _NCB * _HD), bf16, kind="ExternalInput").ap()
    wo = nc.dram_tensor("wo", (128, _HPG * _C), bf16, kind="ExternalInput").ap()
    cs = nc.dram_tensor("cs", (128, 2 * _T), bf16, kind="ExternalInput").ap()
    cst = nc.dram_tensor("cst", (128, 3 * 128), bf16, kind="ExternalInput").ap()
    out = nc.dram_tensor("out", (128, _NT * 4 * _C), bf16, kind="ExternalOutput").ap()

    with tile.TileContext(nc) as tc, ExitStack() as ctx, \
            nc.allow_low_precision(reason="bf16 datapath; f32 psum accumulation"):
        # --- pools ---
        pc = ctx.enter_context(tc.tile_pool(name="pc", bufs=1))         # persistents
        pg = ctx.enter_context(tc.tile_pool(name="pg", bufs=3))         # rope/sq scratch
        pa = ctx.enter_context(tc.tile_pool(name="pa", bufs=_KNOBS["pa_bufs"]))  # A chunks
        psm = ctx.enter_context(tc.tile_pool(name="psm", bufs=3))       # small vectors
        pdb = ctx.enter_context(tc.tile_pool(name="pdb", bufs=2))       # bcast tiles
        pob = ctx.enter_context(tc.tile_pool(name="pob", bufs=2))       # out staging
        # psum pools (8 banks): proj/S x2, sw/c_proj/db x2, O x3, den x1 --
        # this keeps attention-0's accumulators alive while phase A's rope
        # still allocates sw psums.
        ppS = ctx.enter_context(tc.tile_pool(name="ppS", bufs=_KNOBS["ppS"], space="PSUM"))
        ppW = ctx.enter_context(tc.tile_pool(name="ppW", bufs=_KNOBS["ppW"], space="PSUM"))
        ppO = ctx.enter_context(tc.tile_pool(name="ppO", bufs=3, space="PSUM"))
        ppM = ctx.enter_context(tc.tile_pool(name="ppM", bufs=1, space="PSUM"))

        # --- DMA inputs (host pre-packs layouts; all bf16) ---
        # x arrives as 12 half-chunk pieces, T-half-major (host packs DRAM in
        # this exact order): all 6 c-chunks of tiles 0-1 land first (~4.4us)
        # so the tile-0/1 K-projections start early.
        t_xt = pc.tile([128, _NCB * _T], bf16, tag="xt", name="t_xt")
        t_wk = pc.tile([128, _NCB * _HD], bf16, tag="wk", name="t_wk")
        nc.sync.dma_start(t_wk[:], wk[:])
        for j in range(2):
            for c in range(_NCB):
                nc.sync.dma_start(
                    t_xt[:, c * _T + j * 1024: c * _T + (j + 1) * 1024],
                    xT[:, (j * _NCB + c) * 1024: (j * _NCB + c + 1) * 1024])
        t_cs = pc.tile([128, 2 * _T], bf16, tag="cs", name="t_cs")
        nc.sync.dma_start(t_cs[:], cs[:])
        t_cst = pc.tile([128, 3 * 128], bf16, tag="cst", name="t_cst")
        nc.sync.dma_start(t_cst[:], cst[:])
        t_wv = pc.tile([128, _NCB * _HD], bf16, tag="wv", name="t_wv")
        nc.sync.dma_start(t_wv[:], wv[:])
        t_wq = pc.tile([128, _NCB * _HD], bf16, tag="wq", name="t_wq")
        nc.sync.dma_start(t_wq[:], wq[:])
        t_wo = pc.tile([128, _HPG * _C], bf16, tag="wo", name="t_wo")
        nc.sync.dma_start(t_wo[:], wo[:])
        t_eps = pc.tile([128, 1], f32, tag="eps", name="t_eps")
        nc.gpsimd.memset(t_eps[:], _EPS)

        t_tri = t_cst[:, 0:128]
        t_ones_col = t_cst[:, 128:129]
        t_perm = t_cst[:, 256:384]

        # persistent per-head K^T/Q^T (post rope+norm) and V blocks
        t_kn = [pc.tile([128, _T], bf16, tag=f"kn{h}", name=f"kn{h}") for h in range(_HPG)]
        t_qn = [pc.tile([128, _T], bf16, tag=f"qn{h}", name=f"qn{h}") for h in range(_HPG)]
        t_v = [pc.tile([128, _HD], bf16, tag=f"v{tb}", name=f"v{tb}") for tb in range(_NKC)]

        # ---------------- phase A emitters ----------------
        def proj_qk(dst, w, i, h):
            """dst[:, i*512:(i+1)*512] = (W x)^T tile for head h."""
            p = ppS.tile([128, _TW], f32, tag="pS", name="p_qk")
            for c in range(_NCB):
                nc.tensor.matmul(
                    p[:], w[:, c * _HD + h * 128: c * _HD + (h + 1) * 128],
                    t_xt[:, c * _T + i * _TW: c * _T + (i + 1) * _TW],
                    start=(c == 0), stop=(c == _NCB - 1))
            nc.scalar.copy(dst[:, i * _TW:(i + 1) * _TW], p[:])

        def proj_v(tb):
            p = ppS.tile([128, _HD], f32, tag="pS", name="p_v",
                         padded_shape=[128, _TW])
            for c in range(_NCB):
                nc.tensor.matmul(
                    p[:], t_xt[:, c * _T + tb * 128: c * _T + (tb + 1) * 128],
                    t_wv[:, c * _HD:(c + 1) * _HD],
                    start=(c == 0), stop=(c == _NCB - 1))
            nc.scalar.copy(t_v[tb][:], p[:])

        def rope_stage(dst_list, i):
            """In-place RoPE on dst[h][:, i*512:+512] for all heads."""
            isl = slice(i * _TW, (i + 1) * _TW)
            ssl = slice(_T + i * _TW, _T + (i + 1) * _TW)
            sws = []
            for h in range(_HPG):
                p_sw = ppW.tile([128, _TW], f32, tag="pW", name="p_sw")
                nc.tensor.matmul(p_sw[:], t_perm, dst_list[h][:, isl],
                                 start=True, stop=True)
                t_sw = pg.tile([128, _TW], bf16, tag="sw", name="t_sw", bufs=3)
                nc.vector.tensor_mul(t_sw[:], p_sw[:], t_cs[:, ssl])
                sws.append(t_sw)
            for h in range(_HPG):
                nc.vector.tensor_mul(dst_list[h][:, isl], dst_list[h][:, isl],
                                     t_cs[:, isl])
                nc.vector.tensor_add(dst_list[h][:, isl], dst_list[h][:, isl],
                                     sws[h][:])

        def norm_stage(dst_list, i):
            """In-place RMSNorm over partitions (D): square (DVE) ->
            partition all-reduce (GPSIMD) -> sqrt on a (1,512) row (Act) ->
            reciprocal (DVE) -> broadcast (GPSIMD) -> scale (DVE, all-SBUF
            bf16 so it runs in 2x mode)."""
            isl = slice(i * _TW, (i + 1) * _TW)
            sums, bcs = [], []
            for h in range(_HPG):
                t_sq = pg.tile([128, _TW], bf16, tag="sq", name="t_sq", bufs=3)
                nc.vector.tensor_mul(t_sq[:], dst_list[h][:, isl],
                                     dst_list[h][:, isl])
                t_ms = pg.tile([128, _TW], bf16, tag="ms", name="t_ms", bufs=3)
                nc.gpsimd.partition_all_reduce(t_ms[:], t_sq[:], 128, Red.add)
                sums.append(t_ms)
            for h in range(_HPG):
                t_sd = psm.tile([1, _TW], bf16, tag="sd", name="t_sd", bufs=3)
                nc.scalar.activation(t_sd[:], sums[h][0:1, :], Act.Sqrt,
                                     bias=t_eps[0:1, :], scale=1.0 / 128.0)
                t_rs = psm.tile([1, _TW], bf16, tag="rs", name="t_rs", bufs=3)
                nc.vector.reciprocal(t_rs[:], t_sd[:])
                t_bc = pdb.tile([128, _TW], bf16, tag="bc", name="t_bc")
                nc.gpsimd.partition_broadcast(t_bc[:], t_rs[:])
                bcs.append(t_bc)
            for h in range(_HPG):
                nc.vector.tensor_mul(dst_list[h][:, isl], dst_list[h][:, isl],
                                     bcs[h][:])

        # ---------------- phase B ----------------
        def attention_rounds(qt, split_acc=False):
            """Causal attention for all 3 heads of one T_q tile, heads
            round-robin per k-chunk. Returns (round_closures, finisher);
            the finisher returns the z tiles."""
            nchunk = 4 * qt + 4
            LOOKAHEAD = _KNOBS["lookahead"]
            st = {}

            st["a"] = {}

            def init():
                st["den"] = ppM.tile([65, _TW], f32, tag="pM", name="p_den")
                st["os"] = [ppO.tile([128, _TW], f32, tag="pO", name=f"p_o{h}")
                            for h in range(_HPG)]

            def emit_s(kc, h):
                roff = 0 if kc < 4 * qt else (kc - 4 * qt) * 128
                nsl = slice(roff, _TW)
                ksl = slice(kc * 128, (kc + 1) * 128)
                p_s = ppS.tile([128, _TW], f32, tag="pS", name="p_s")
                nc.tensor.matmul(p_s[:, nsl], t_kn[h][:, ksl],
                                 t_qn[h][:, qt * _TW + roff:(qt + 1) * _TW],
                                 start=True, stop=True)
                t_a = pa.tile([128, _TW], bf16, tag="a", name="t_a")
                nc.scalar.activation(t_a[:, nsl], p_s[:, nsl], Act.Exp,
                                     scale=1.0 / float(np.sqrt(_D)))
                if kc >= 4 * qt:  # diagonal chunk: triangular mask
                    dsl = slice(roff, roff + 128)
                    nc.gpsimd.tensor_mul(t_a[:, dsl], t_a[:, dsl], t_tri)
                st["a"][(kc, h)] = t_a

            def emit_acc(kc, h):
                roff = 0 if kc < 4 * qt else (kc - 4 * qt) * 128
                nsl = slice(roff, _TW)
                t_a = st["a"].pop((kc, h))
                nc.tensor.matmul(st["den"][32 * h:32 * h + 1, nsl], t_ones_col,
                                 t_a[:, nsl],
                                 start=(kc == 0), stop=(kc == nchunk - 1))
                nc.tensor.matmul(st["os"][h][:, nsl],
                                 t_v[kc][:, h * 128:(h + 1) * 128], t_a[:, nsl],
                                 start=(kc == 0), stop=(kc == nchunk - 1))

            def make_round(kc):
                def r():
                    if kc == 0:
                        init()
                    for h in range(_HPG):
                        if kc < nchunk:
                            emit_s(kc, h)
                        if kc >= LOOKAHEAD:
                            emit_acc(kc - LOOKAHEAD, h)
                return r

            def make_s_round(kc):
                def r():
                    for h in range(_HPG):
                        emit_s(kc, h)
                return r

            def make_acc_round(kc):
                def r():
                    if kc == 0:
                        init()
                    for h in range(_HPG):
                        emit_acc(kc, h)
                return r

            def finisher():
                # extract dens, PE broadcast (partition_broadcast cannot read
                # partition-offset rows), reciprocal, mul
                t_dn = psm.tile([65, _TW], bf16, tag="dn", name="t_dn", bufs=3)
                nc.scalar.copy(t_dn[:], st["den"][:])
                zs = []
                for h in range(_HPG):
                    p_db = ppW.tile([128, _TW], f32, tag="pW", name="p_db")
                    nc.tensor.matmul(p_db[:], t_cst[32 * h:32 * h + 1, 128:256],
                                     t_dn[32 * h:32 * h + 1, :],
                                     start=True, stop=True)
                    t_db = pdb.tile([128, _TW], bf16, tag="db", name="t_db")
                    nc.vector.reciprocal(t_db[:], p_db[:])
                    t_z = pc.tile([128, _TW], bf16, tag=f"z{h}_{qt % 2}",
                                  name=f"z{h}_{qt % 2}")
                    nc.vector.tensor_mul(t_z[:], st["os"][h][:], t_db[:])
                    zs.append(t_z)
                return zs

            if split_acc:
                # S/exp rounds separated from pure-PE den/AV rounds: the
                # S rounds ride along phase A; the accs fill phase B windup.
                s_rounds = [make_s_round(kc) for kc in range(nchunk)]
                acc_rounds = [make_acc_round(kc) for kc in range(nchunk)]
                return s_rounds, acc_rounds, finisher
            rounds = [make_round(kc) for kc in range(nchunk + _KNOBS["lookahead"])]
            return rounds, finisher

        def attention(qt):
            rounds, fin = attention_rounds(qt)
            for r in rounds:
                r()
            return fin()

        _ob = {}

        def c_proj_half(qt, zs, blks):
            if qt not in _ob:
                _ob[qt] = pob.tile([128, 4 * _C], bf16, tag="ob", name="t_ob")
            t_ob = _ob[qt]
            for blk in blks:
                bsl = slice(blk * 128, (blk + 1) * 128)
                for nh in range(2):
                    p_c = ppW.tile([128, 384], f32, tag="pW", name="p_c",
                                   padded_shape=[128, _TW])
                    for hh in range(_HPG):
                        nc.tensor.matmul(
                            p_c[:], zs[hh][:, bsl],
                            t_wo[:, hh * _C + nh * 384: hh * _C + (nh + 1) * 384],
                            start=(hh == 0), stop=(hh == _HPG - 1))
                    o0 = blk * _C + nh * 384
                    nc.vector.tensor_copy(t_ob[:, o0:o0 + 384], p_c[:])
            if 3 in blks:
                nc.sync.dma_start(out[:, qt * 4 * _C:(qt + 1) * 4 * _C], t_ob[:])

        # ---------------- emission schedule ----------------
        # Phase A: K projs (DMA-paced), then per tile rope/norm of K and Q
        # with a fill queue of independent PE work in the dependency windows:
        # V blocks 0..7 first, then attention(0)'s chunk rounds (it only
        # needs K/Q tile 0 + V blocks 0..3). V blocks 8..15 are deferred to
        # phase B (their extract is Copy = Exp-table, no act-table switch)
        # where they fill the qt=1 pipeline windup.
        for i in range(_NT):
            for h in range(_HPG):
                proj_qk(t_kn[h], t_wk, i, h)
        fillq = [lambda tb=tb: (proj_v(2 * tb), proj_v(2 * tb + 1))
                 for tb in range(8)]             # V pairs 0..15
        fin0 = None

        def fill(n):
            for _ in range(n):
                if fillq:
                    fillq.pop(0)()

        att1 = _KNOBS.get("att1_overlap", False)
        acc1 = fin1 = None
        for i in range(_NT):
            rope_stage(t_kn, i)
            fill(2)
            norm_stage(t_kn, i)
            for h in range(_HPG):                # PE filler while K norm runs
                proj_qk(t_qn[h], t_wq, i, h)
            fill(1)
            rope_stage(t_qn, i)
            fill(1)
            norm_stage(t_qn, i)
            if i == 0:
                rounds0, fin0 = attention_rounds(0)
                if _KNOBS["att0_overlap"]:
                    fillq.extend(rounds0)
                else:
                    deferred0 = list(rounds0)
            else:
                fill(1)
            if i == 1 and att1:
                s1, acc1, fin1 = attention_rounds(1, split_acc=True)
                fillq.extend(s1)
        fill(len(fillq))
        if not _KNOBS["att0_overlap"]:
            for r in deferred0:
                r()
        prev = (0, fin0())
        for qt in range(1, _NT):
            pqt, pzs = prev
            if qt == 1 and att1:
                # att1's S/exp already ran in phase A; its den/AV rounds are
                # pure PE and fill the windup here.
                for r in acc1:
                    r()
                c_proj_half(pqt, pzs, [0, 1])
                c_proj_half(pqt, pzs, [2, 3])
                prev = (1, fin1())
                continue
            rounds, fin = attention_rounds(qt)
            c_proj_half(pqt, pzs, [0, 1])
            c_proj_half(pqt, pzs, [2, 3])
            for r in rounds:
                r()
            prev = (qt, fin())
        qt, zs = prev
        c_proj_half(qt, zs, [0, 1])
        c_proj_half(qt, zs, [2, 3])

    nc.compile()
    return nc


def _get_nc():
    if "nc" not in _cached:
        _cached["nc"] = _build_nc()
    return _cached["nc"]


def _bf16(a):
    return np.ascontiguousarray(a.astype(ml_dtypes.bfloat16))


def make_in_maps(x, cos, sin, Wq, Wk, Wv, Wo):
    cosT = cos.reshape(_T, _D // 2).T                        # (64, T)
    sinT = sin.reshape(_T, _D // 2).T
    cc = np.concatenate([cosT, cosT], axis=0)                # (128, T)
    ss = np.concatenate([sinT, -sinT], axis=0)
    cs = _bf16(np.concatenate([cc, ss], axis=1))             # (128, 2T)
    tri = (np.arange(128)[None, :] >= np.arange(128)[:, None]).astype(np.float32)
    ones128 = np.ones((128, 128), dtype=np.float32)
    permm = np.zeros((128, 128), dtype=np.float32)           # half-swap permutation
    for d in range(64):
        permm[64 + d, d] = 1.0
        permm[d, 64 + d] = 1.0
    cst = _bf16(np.concatenate([tri, ones128, permm], axis=1))

    def pack_w(w):  # (768, 384) -> (128, 2304) c-chunk-major
        return _bf16(w.reshape(_NCB, 128, _HD).transpose(1, 0, 2).reshape(128, -1))

    def pack_wo(w):  # (384, 768) -> (128, 2304) head-chunk-major
        return _bf16(w.reshape(_HPG, 128, _C).transpose(1, 0, 2).reshape(128, -1))

    in_maps = []
    for core in range(8):
        b, g = divmod(core, 2)
        gsl = slice(g * _HD, (g + 1) * _HD)
        # (128, 12288) in DMA piece order: T-half-major, then c-chunk
        xt = (x[b].T.reshape(_NCB, 128, 2, 1024).transpose(1, 2, 0, 3)
              .reshape(128, -1))
        in_maps.append({
            "xT": _bf16(xt),
            "wq": pack_w(Wq[gsl, :].T),
            "wk": pack_w(Wk[gsl, :].T),
            "wv": pack_w(Wv[gsl, :].T),
            "wo": pack_wo(Wo[:, gsl].T),
            "cs": cs, "cst": cst,
        })
    return in_maps


def unshard(core_outs):
    """core_outs: list of 8 per-core (128, 12288) bf16 arrays -> (4, 2048, 768)."""
    full = []
    for b in range(_B):
        acc = None
        for g in range(2):
            dev = np.asarray(core_outs[2 * b + g]).astype(np.float32)
            part = dev.reshape(128, _NT, 4, _C).transpose(1, 2, 0, 3).reshape(_T, _C)
            acc = part if acc is None else acc + part
        full.append(acc)
    return np.stack(full, axis=0)


def kernel(x, cos, sin, Wq, Wk, Wv, Wo):
    from concourse.bass_utils import run_bass_kernel_spmd

    x = np.asarray(x, dtype=np.float32)
    cos = np.asarray(cos, dtype=np.float32)
    sin = np.asarray(sin, dtype=np.float32)
    Wq = np.asarray(Wq, dtype=np.float32)
    Wk = np.asarray(Wk, dtype=np.float32)
    Wv = np.asarray(Wv, dtype=np.float32)
    Wo = np.asarray(Wo, dtype=np.float32)

    nc = _get_nc()
    in_maps = make_in_maps(x, cos, sin, Wq, Wk, Wv, Wo)
    res = run_bass_kernel_spmd(nc, in_maps, core_ids=list(range(8)))
    return unshard([r_["out"] for r_ in res.results])
